# revision 3
# baseline (speedup 1.0000x reference)
"""Trainium2 Bass kernel for the restoring-division iteration (48-bit SNN bit vectors).

Full inputs R, D_extended: [1_000_000, 48] float32 bits (LSB first).
Outputs: Q_bit [1_000_000, 1], R_next [1_000_000, 48] float32.

Strategy: pure data parallel over 8 NeuronCores (125_000 rows each).
Per core the row-local 48-bit ripple-carry subtract A + ~B + 1 is computed
with a single DVE tensor_tensor_scan per tile (carry recurrence
c' = p*c + g), using a separator column between rows to reset the carry.

  d = R - D            in {-1, 0, 1}   (GPSIMD, into 49-wide padded layout)
  p = (d == 0)         XNOR(a,b)  = carry propagate
  g = (d >= 0.5)       a AND ~b   = carry generate
  sep column: d = 1 -> p = 0, g = 1 -> scan state resets to 1 (the +1 of
  two's complement) at each row boundary.
  carry scan: c[t] = p[t]*c[t-1] + g[t]
  trial bits: s_i = p_i XOR c_{i-1}  (s_0 = NOT p_0)
  Q_bit = carry out of bit 47; R_next = Q ? s : R (copy_predicated).
"""

import sys

for _p in ("/opt/trn_rl_repo",):
    if _p not in sys.path:
        sys.path.insert(0, _p)

import numpy as np

N_CORES = 8
B_TOTAL = 1_000_000
BITS = 48
ROWS_PER_CORE = B_TOTAL // N_CORES  # 125_000

# Per-core tiling: P partitions x F rows per partition per tile, NT tiles.
P = 125
F = 100
NT = ROWS_PER_CORE // (P * F)  # 10
assert P * F * NT == ROWS_PER_CORE

K = BITS  # 48
KP = K + 1  # 49, padded row with separator column

_cache = {}


def _build_program():
    import concourse.bacc as bacc
    import concourse.mybir as mybir
    import concourse.tile as tile
    from concourse._compat import with_exitstack
    from contextlib import ExitStack

    f32 = mybir.dt.float32
    bf16 = mybir.dt.bfloat16
    Op = mybir.AluOpType

    nc = bacc.Bacc(
        "TRN2",
        target_bir_lowering=False,
        debug=False,
        enable_asserts=False,
        num_devices=N_CORES,
    )

    R_d = nc.dram_tensor("R", [ROWS_PER_CORE, K], f32, kind="ExternalInput").ap()
    D_d = nc.dram_tensor(
        "D_extended", [ROWS_PER_CORE, K], f32, kind="ExternalInput"
    ).ap()
    Q_d = nc.dram_tensor("Q_bit", [ROWS_PER_CORE, 1], f32, kind="ExternalOutput").ap()
    Rn_d = nc.dram_tensor(
        "R_next", [ROWS_PER_CORE, K], f32, kind="ExternalOutput"
    ).ap()

    # [ROWS, K] -> [NT, P, F*K] with partition-contiguous row blocks
    R_v = R_d.rearrange("(t p f) k -> t p (f k)", t=NT, p=P, f=F)
    D_v = D_d.rearrange("(t p f) k -> t p (f k)", t=NT, p=P, f=F)
    Rn_v = Rn_d.rearrange("(t p f) k -> t p (f k)", t=NT, p=P, f=F)
    Q_v = Q_d.rearrange("(t p f) k -> t p (f k)", t=NT, p=P, f=F)

    with ExitStack() as ctx:
        tc = ctx.enter_context(tile.TileContext(nc))
        rio = ctx.enter_context(tc.tile_pool(name="rio", bufs=2))
        din = ctx.enter_context(tc.tile_pool(name="din", bufs=2))
        dpool = ctx.enter_context(tc.tile_pool(name="dpool", bufs=2))
        ppool = ctx.enter_context(tc.tile_pool(name="ppool", bufs=2))
        gpool = ctx.enter_context(tc.tile_pool(name="gpool", bufs=2))
        cpool = ctx.enter_context(tc.tile_pool(name="cpool", bufs=2))
        qpool = ctx.enter_context(tc.tile_pool(name="qpool", bufs=2))
        mpool = ctx.enter_context(tc.tile_pool(name="mpool", bufs=2))

        for t in range(NT):
            Rt = rio.tile([P, F * K], f32)
            nc.sync.dma_start(Rt[:, :], R_v[t])
            Dt = din.tile([P, F * K], f32)
            nc.sync.dma_start(Dt[:, :], D_v[t])

            Rt3 = Rt[:, :].rearrange("p (f k) -> p f k", k=K)
            Dt3 = Dt[:, :].rearrange("p (f k) -> p f k", k=K)

            d49 = dpool.tile([P, F * KP], f32)
            d3 = d49[:, :].rearrange("p (f k) -> p f k", k=KP)
            # d = R - D into bit columns; separator column = 1.0
            nc.gpsimd.tensor_tensor(d3[:, :, 0:K], Rt3, Dt3, op=Op.subtract)
            nc.gpsimd.memset(d3[:, :, K : K + 1], 1.0)

            p49 = ppool.tile([P, F * KP], bf16)
            nc.vector.tensor_scalar(p49[:, :], d49[:, :], 0.0, None, op0=Op.is_equal)
            g49 = gpool.tile([P, F * KP], bf16)
            nc.vector.tensor_scalar(g49[:, :], d49[:, :], 0.5, None, op0=Op.is_ge)

            cbuf = cpool.tile([P, F * KP], bf16)
            nc.vector.tensor_tensor_scan(
                cbuf[:, :], p49[:, :], g49[:, :], 1.0, op0=Op.mult, op1=Op.add
            )

            p3 = p49[:, :].rearrange("p (f k) -> p f k", k=KP)
            g3 = g49[:, :].rearrange("p (f k) -> p f k", k=KP)
            c3 = cbuf[:, :].rearrange("p (f k) -> p f k", k=KP)

            # trial sum bits, written over g (dead after the scan):
            # s_i = p_i XOR c_{i-1} for i>=1; s_0 = NOT p_0
            nc.vector.tensor_tensor(
                g3[:, :, 1:K], p3[:, :, 1:K], c3[:, :, 0 : K - 1], op=Op.logical_xor
            )
            nc.vector.tensor_scalar(
                g3[:, :, 0:1], p3[:, :, 0:1], -1.0, 1.0, op0=Op.mult, op1=Op.add
            )

            # Q = carry out of bit 47; where Q==1 accept trial bits, else keep R.
            # copy_predicated wants an integer mask -> compact carry-out column
            # to an int8 tile, broadcast over the bit dim.
            mt = mpool.tile([P, F], mybir.dt.int8)
            nc.vector.tensor_scalar(
                mt[:, :], c3[:, :, K - 1 : K], 0.5, None, op0=Op.is_ge
            )
            mask = mt[:, :].unsqueeze(2).broadcast_to([P, F, K])
            nc.vector.copy_predicated(Rt3, mask, g3[:, :, 0:K])

            Qt = qpool.tile([P, F], f32)
            nc.scalar.copy(Qt[:, :], c3[:, :, K - 1 : K])

            nc.sync.dma_start(Rn_v[t], Rt[:, :])
            nc.sync.dma_start(Q_v[t], Qt[:, :])

    nc.compile()
    return nc


def _get_program():
    if "nc" not in _cache:
        _cache["nc"] = _build_program()
    return _cache["nc"]


def kernel(R: np.ndarray, D_extended: np.ndarray):
    from concourse.bass_utils import run_bass_kernel_spmd

    nc = _get_program()

    R = np.ascontiguousarray(R, dtype=np.float32)
    D = np.ascontiguousarray(D_extended, dtype=np.float32)
    in_maps = [
        {
            "R": R[c * ROWS_PER_CORE : (c + 1) * ROWS_PER_CORE],
            "D_extended": D[c * ROWS_PER_CORE : (c + 1) * ROWS_PER_CORE],
        }
        for c in range(N_CORES)
    ]
    res = run_bass_kernel_spmd(nc, in_maps, list(range(N_CORES))).results
    Q = np.concatenate([r["Q_bit"] for r in res], axis=0)
    Rn = np.concatenate([r["R_next"] for r in res], axis=0)
    return Q, Rn


# revision 6
# speedup vs baseline: 1.0444x; 1.0444x over previous
"""Trainium2 Bass kernel for the restoring-division iteration (48-bit SNN bit vectors).

Full inputs R, D_extended: [1_000_000, 48] float32 bits (LSB first).
Outputs: Q_bit [1_000_000, 1], R_next [1_000_000, 48] float32.

Strategy: pure data parallel over 8 NeuronCores (125_000 rows each).
Per core the row-local 48-bit ripple-carry subtract A + ~B + 1 is computed
with a single DVE tensor_tensor_scan per tile (carry recurrence
c' = p*c + g), using a separator column between rows to reset the carry.

  d = R - D            in {-1, 0, 1}   (GPSIMD, into 49-wide padded layout)
  p = (d == 0)         XNOR(a,b)  = carry propagate
  g = (d >= 0.5)       a AND ~b   = carry generate
  sep column: d = 1 -> p = 0, g = 1 -> scan state resets to 1 (the +1 of
  two's complement) at each row boundary.
  carry scan: c[t] = p[t]*c[t-1] + g[t]
  trial bits: s_i = p_i XOR c_{i-1}  (s_0 = NOT p_0)
  Q_bit = carry out of bit 47; R_next = Q ? s : R (copy_predicated).
"""

import sys

for _p in ("/opt/trn_rl_repo",):
    if _p not in sys.path:
        sys.path.insert(0, _p)

import numpy as np

N_CORES = 8
B_TOTAL = 1_000_000
BITS = 48
ROWS_PER_CORE = B_TOTAL // N_CORES  # 125_000

# Per-core tiling: P partitions x F rows per partition per tile, NT tiles.
P = 125
F = 100
NT = ROWS_PER_CORE // (P * F)  # 10
assert P * F * NT == ROWS_PER_CORE

K = BITS  # 48
KP = K + 1  # 49, padded row with separator column

_cache = {}


def _build_program():
    import concourse.bacc as bacc
    import concourse.mybir as mybir
    import concourse.tile as tile
    from concourse._compat import with_exitstack
    from contextlib import ExitStack

    f32 = mybir.dt.float32
    bf16 = mybir.dt.bfloat16
    Op = mybir.AluOpType

    nc = bacc.Bacc(
        "TRN2",
        target_bir_lowering=False,
        debug=False,
        enable_asserts=False,
        num_devices=N_CORES,
    )

    R_d = nc.dram_tensor("R", [ROWS_PER_CORE, K], f32, kind="ExternalInput").ap()
    D_d = nc.dram_tensor(
        "D_extended", [ROWS_PER_CORE, K], f32, kind="ExternalInput"
    ).ap()
    Q_d = nc.dram_tensor("Q_bit", [ROWS_PER_CORE, 1], f32, kind="ExternalOutput").ap()
    Rn_d = nc.dram_tensor(
        "R_next", [ROWS_PER_CORE, K], f32, kind="ExternalOutput"
    ).ap()

    # [ROWS, K] -> [NT, P, F*K] with partition-contiguous row blocks
    R_v = R_d.rearrange("(t p f) k -> t p (f k)", t=NT, p=P, f=F)
    D_v = D_d.rearrange("(t p f) k -> t p (f k)", t=NT, p=P, f=F)
    Rn_v = Rn_d.rearrange("(t p f) k -> t p (f k)", t=NT, p=P, f=F)
    Q_v = Q_d.rearrange("(t p f) k -> t p (f k)", t=NT, p=P, f=F)

    with ExitStack() as ctx:
        tc = ctx.enter_context(tile.TileContext(nc))
        rio = ctx.enter_context(tc.tile_pool(name="rio", bufs=3))
        din = ctx.enter_context(tc.tile_pool(name="din", bufs=2))
        dpool = ctx.enter_context(tc.tile_pool(name="dpool", bufs=2))
        ppool = ctx.enter_context(tc.tile_pool(name="ppool", bufs=2))
        gpool = ctx.enter_context(tc.tile_pool(name="gpool", bufs=2))
        cpool = ctx.enter_context(tc.tile_pool(name="cpool", bufs=2))
        qpool = ctx.enter_context(tc.tile_pool(name="qpool", bufs=2))
        mpool = ctx.enter_context(tc.tile_pool(name="mpool", bufs=2))

        for t in range(NT):
            # Split traffic across the three descriptor-gen paths so all
            # SDMA engine groups run concurrently: R-load on sync HWDGE,
            # D-load on scalar HWDGE, R_next-store on gpsimd SWDGE.
            Rt = rio.tile([P, F * K], f32)
            nc.sync.dma_start(Rt[:, :], R_v[t])
            Dt = din.tile([P, F * K], f32)
            nc.scalar.dma_start(Dt[:, :], D_v[t])

            Rt3 = Rt[:, :].rearrange("p (f k) -> p f k", k=K)
            Dt3 = Dt[:, :].rearrange("p (f k) -> p f k", k=K)

            d49 = dpool.tile([P, F * KP], bf16)
            d3 = d49[:, :].rearrange("p (f k) -> p f k", k=KP)
            # d = R - D into bit columns; separator column = 1.0
            nc.gpsimd.tensor_tensor(d3[:, :, 0:K], Rt3, Dt3, op=Op.subtract)
            nc.gpsimd.memset(d3[:, :, K : K + 1], 1.0)

            p49 = ppool.tile([P, F * KP], bf16)
            nc.vector.tensor_scalar(p49[:, :], d49[:, :], 0.0, None, op0=Op.is_equal)
            # g = (d >= 0.5) == relu(d) on {-1,0,1}: compute on the idle ACT engine
            g49 = gpool.tile([P, F * KP], bf16)
            nc.scalar.activation(
                g49[:, :], d49[:, :], mybir.ActivationFunctionType.Relu
            )

            cbuf = cpool.tile([P, F * KP], bf16)
            nc.vector.tensor_tensor_scan(
                cbuf[:, :], p49[:, :], g49[:, :], 1.0, op0=Op.mult, op1=Op.add
            )

            p3 = p49[:, :].rearrange("p (f k) -> p f k", k=KP)
            g3 = g49[:, :].rearrange("p (f k) -> p f k", k=KP)
            c3 = cbuf[:, :].rearrange("p (f k) -> p f k", k=KP)

            # trial sum bits, written over g (dead after the scan):
            # s_i = p_i XOR c_{i-1} for i>=1; s_0 = NOT p_0
            nc.vector.tensor_tensor(
                g3[:, :, 1:K], p3[:, :, 1:K], c3[:, :, 0 : K - 1], op=Op.logical_xor
            )
            nc.vector.tensor_scalar(
                g3[:, :, 0:1], p3[:, :, 0:1], -1.0, 1.0, op0=Op.mult, op1=Op.add
            )

            # Q = carry out of bit 47; where Q==1 accept trial bits, else keep R.
            # copy_predicated wants an integer mask -> compact carry-out column
            # to an int8 tile, broadcast over the bit dim.
            mt = mpool.tile([P, F], mybir.dt.int8)
            nc.vector.tensor_scalar(
                mt[:, :], c3[:, :, K - 1 : K], 0.5, None, op0=Op.is_ge
            )
            mask = mt[:, :].unsqueeze(2).broadcast_to([P, F, K])
            nc.vector.copy_predicated(Rt3, mask, g3[:, :, 0:K])

            Qt = qpool.tile([P, F], f32)
            nc.scalar.copy(Qt[:, :], c3[:, :, K - 1 : K])

            nc.gpsimd.dma_start(Rn_v[t], Rt[:, :])
            nc.sync.dma_start(Q_v[t], Qt[:, :])

    nc.compile()
    return nc


def _get_program():
    if "nc" not in _cache:
        _cache["nc"] = _build_program()
    return _cache["nc"]


def kernel(R: np.ndarray, D_extended: np.ndarray):
    from concourse.bass_utils import run_bass_kernel_spmd

    nc = _get_program()

    R = np.ascontiguousarray(R, dtype=np.float32)
    D = np.ascontiguousarray(D_extended, dtype=np.float32)
    in_maps = [
        {
            "R": R[c * ROWS_PER_CORE : (c + 1) * ROWS_PER_CORE],
            "D_extended": D[c * ROWS_PER_CORE : (c + 1) * ROWS_PER_CORE],
        }
        for c in range(N_CORES)
    ]
    res = run_bass_kernel_spmd(nc, in_maps, list(range(N_CORES))).results
    Q = np.concatenate([r["Q_bit"] for r in res], axis=0)
    Rn = np.concatenate([r["R_next"] for r in res], axis=0)
    return Q, Rn


# revision 11
# speedup vs baseline: 1.1736x; 1.1238x over previous
"""Trainium2 Bass kernel for the restoring-division iteration (48-bit SNN bit vectors).

Full inputs R, D_extended: [1_000_000, 48] float32 bits (LSB first).
Outputs: Q_bit [1_000_000, 1], R_next [1_000_000, 48] float32.

Strategy: pure data parallel over 8 NeuronCores (125_000 rows each).
Per core the row-local 48-bit ripple-carry subtract A + ~B + 1 is computed
with a single DVE tensor_tensor_scan per tile (carry recurrence
c' = p*c + g), using a separator column between rows to reset the carry.

  d = R - D            in {-1, 0, 1}   (GPSIMD, into 49-wide padded layout)
  p = (d == 0)         XNOR(a,b)  = carry propagate
  g = (d >= 0.5)       a AND ~b   = carry generate
  sep column: d = 1 -> p = 0, g = 1 -> scan state resets to 1 (the +1 of
  two's complement) at each row boundary.
  carry scan: c[t] = p[t]*c[t-1] + g[t]
  trial bits: s_i = p_i XOR c_{i-1}  (s_0 = NOT p_0)
  Q_bit = carry out of bit 47; R_next = Q ? s : R (copy_predicated).
"""

import sys

for _p in ("/opt/trn_rl_repo",):
    if _p not in sys.path:
        sys.path.insert(0, _p)

import numpy as np

N_CORES = 8
B_TOTAL = 1_000_000
BITS = 48
ROWS_PER_CORE = B_TOTAL // N_CORES  # 125_000

# Per-core tiling: P partitions x F rows per partition per tile, NT tiles.
P = 125
F = 100
NT = ROWS_PER_CORE // (P * F)  # 10
assert P * F * NT == ROWS_PER_CORE

K = BITS  # 48
KP = K + 1  # 49, padded row with separator column

_cache = {}


def _build_program():
    import concourse.bacc as bacc
    import concourse.mybir as mybir
    import concourse.tile as tile
    from concourse._compat import with_exitstack
    from contextlib import ExitStack

    f32 = mybir.dt.float32
    bf16 = mybir.dt.bfloat16
    Op = mybir.AluOpType

    nc = bacc.Bacc(
        "TRN2",
        target_bir_lowering=False,
        debug=False,
        enable_asserts=False,
        num_devices=N_CORES,
    )

    R_d = nc.dram_tensor("R", [ROWS_PER_CORE, K], f32, kind="ExternalInput").ap()
    D_d = nc.dram_tensor(
        "D_extended", [ROWS_PER_CORE, K], f32, kind="ExternalInput"
    ).ap()
    Q_d = nc.dram_tensor("Q_bit", [ROWS_PER_CORE, 1], f32, kind="ExternalOutput").ap()
    Rn_d = nc.dram_tensor(
        "R_next", [ROWS_PER_CORE, K], f32, kind="ExternalOutput"
    ).ap()

    # [ROWS, K] -> [NT, P, F*K] with partition-contiguous row blocks
    R_v = R_d.rearrange("(t p f) k -> t p (f k)", t=NT, p=P, f=F)
    D_v = D_d.rearrange("(t p f) k -> t p (f k)", t=NT, p=P, f=F)
    Rn_v = Rn_d.rearrange("(t p f) k -> t p (f k)", t=NT, p=P, f=F)
    Q_v = Q_d.rearrange("(t p f) k -> t p (f k)", t=NT, p=P, f=F)

    with ExitStack() as ctx:
        tc = ctx.enter_context(tile.TileContext(nc))
        rio = ctx.enter_context(tc.tile_pool(name="rio", bufs=2))
        din = ctx.enter_context(tc.tile_pool(name="din", bufs=2))
        dpool = ctx.enter_context(tc.tile_pool(name="dpool", bufs=2))
        ppool = ctx.enter_context(tc.tile_pool(name="ppool", bufs=2))
        gpool = ctx.enter_context(tc.tile_pool(name="gpool", bufs=2))
        cpool = ctx.enter_context(tc.tile_pool(name="cpool", bufs=2))
        qpool = ctx.enter_context(tc.tile_pool(name="qpool", bufs=2))
        mpool = ctx.enter_context(tc.tile_pool(name="mpool", bufs=2))

        for t in range(NT):
            # The two HWDGE rings (sync/scalar) share one 5-engine SDMA
            # group; the gpsimd SWDGE ring stripes across all 16 SDMA
            # engines. Route all bulk traffic through SWDGE; only the tiny
            # Q-store uses HWDGE. The D-load casts to bf16 in the SDMA
            # datapath (values are 0/1 bits - exact) to save SBUF.
            Rt = rio.tile([P, F * K], f32)
            nc.gpsimd.dma_start(Rt[:, :], R_v[t])
            Dt = din.tile([P, F * K], f32)
            nc.gpsimd.dma_start(Dt[:, :], D_v[t])

            Rt3 = Rt[:, :].rearrange("p (f k) -> p f k", k=K)
            Dt3 = Dt[:, :].rearrange("p (f k) -> p f k", k=K)

            d49 = dpool.tile([P, F * KP], bf16)
            d3 = d49[:, :].rearrange("p (f k) -> p f k", k=KP)
            # d = R - D into bit columns; separator column = 1.0
            nc.gpsimd.tensor_tensor(d3[:, :, 0:K], Rt3, Dt3, op=Op.subtract)
            nc.gpsimd.memset(d3[:, :, K : K + 1], 1.0)

            p49 = ppool.tile([P, F * KP], bf16)
            nc.vector.tensor_scalar(p49[:, :], d49[:, :], 0.0, None, op0=Op.is_equal)
            # g = (d >= 0.5) == relu(d) on {-1,0,1}: compute on the idle ACT engine
            g49 = gpool.tile([P, F * KP], bf16)
            nc.scalar.activation(
                g49[:, :], d49[:, :], mybir.ActivationFunctionType.Relu
            )

            cbuf = cpool.tile([P, F * KP], bf16)
            nc.vector.tensor_tensor_scan(
                cbuf[:, :], p49[:, :], g49[:, :], 1.0, op0=Op.mult, op1=Op.add
            )

            p3 = p49[:, :].rearrange("p (f k) -> p f k", k=KP)
            g3 = g49[:, :].rearrange("p (f k) -> p f k", k=KP)
            c3 = cbuf[:, :].rearrange("p (f k) -> p f k", k=KP)

            # trial sum bits, written over g (dead after the scan):
            # s_i = p_i XOR c_{i-1} for i>=1; s_0 = NOT p_0
            nc.vector.tensor_tensor(
                g3[:, :, 1:K], p3[:, :, 1:K], c3[:, :, 0 : K - 1], op=Op.logical_xor
            )
            nc.vector.tensor_scalar(
                g3[:, :, 0:1], p3[:, :, 0:1], -1.0, 1.0, op0=Op.mult, op1=Op.add
            )

            # Q = carry out of bit 47; where Q==1 accept trial bits, else keep R.
            # copy_predicated wants an integer mask -> compact carry-out column
            # to an int8 tile, broadcast over the bit dim.
            mt = mpool.tile([P, F], mybir.dt.int8)
            nc.vector.tensor_scalar(
                mt[:, :], c3[:, :, K - 1 : K], 0.5, None, op0=Op.is_ge
            )
            mask = mt[:, :].unsqueeze(2).broadcast_to([P, F, K])
            nc.vector.copy_predicated(Rt3, mask, g3[:, :, 0:K])

            Qt = qpool.tile([P, F], f32)
            nc.scalar.copy(Qt[:, :], c3[:, :, K - 1 : K])

            nc.gpsimd.dma_start(Rn_v[t], Rt[:, :])
            nc.sync.dma_start(Q_v[t], Qt[:, :])

    nc.compile()
    return nc


def _get_program():
    if "nc" not in _cache:
        _cache["nc"] = _build_program()
    return _cache["nc"]


def kernel(R: np.ndarray, D_extended: np.ndarray):
    from concourse.bass_utils import run_bass_kernel_spmd

    nc = _get_program()

    R = np.ascontiguousarray(R, dtype=np.float32)
    D = np.ascontiguousarray(D_extended, dtype=np.float32)
    in_maps = [
        {
            "R": R[c * ROWS_PER_CORE : (c + 1) * ROWS_PER_CORE],
            "D_extended": D[c * ROWS_PER_CORE : (c + 1) * ROWS_PER_CORE],
        }
        for c in range(N_CORES)
    ]
    res = run_bass_kernel_spmd(nc, in_maps, list(range(N_CORES))).results
    Q = np.concatenate([r["Q_bit"] for r in res], axis=0)
    Rn = np.concatenate([r["R_next"] for r in res], axis=0)
    return Q, Rn
